# revision 36
# baseline (speedup 1.0000x reference)
"""LIF spiking-neuron kernel for Trainium2 (Bass/Tile), 8-core data-parallel.

Reference semantics (per element, scan over T=8):
    mem = mem * 0.5 + x_t
    s_t = (mem > 1.0) ? 1.0 : 0.0        # forward value of the spike
    mem = mem - s_t

Spikes leave the device as ONE byte per element: the ACT engine writes
sg_t = fp8e4(Sign(mem_t - thr)) in a single op ({-1,0,+1}; no Relu) and
the host decodes spikes as (byte == 0x38), the fp8 encoding of +1.0.
This cuts HBM store traffic 4x vs fp32 (16.78 MB in + 4.19 MB out per
core), so the Vector engine and the load stream set the pace.

The free axis (4096 fp32/partition) advances through the T=8 scan as
three column groups per step, split across two engine pipelines so the
Vector engine (the 2-tensor-op bottleneck at ~123 G elem/s) tracks the
DMA load rate:

  PE-assisted pair (cols 0:2048 as two 1024 chunks, membrane p in SBUF):
      PE:   bank_{t+1} = I @ x_{t+1}            # fp32 matmul, start=True
      ACT:  sg_t = fp8(Sign(p_t - (1+c_t)))     # also the output tile
      PE:   bank_{t+1} += (-0.25 I) @ sg_t      # fp8 matmul, stop=True
      DVE:  p_{t+1} = 0.5 * p_t + bank_{t+1}    # one stt (SBUF + PSUM in)
  where p_t = mem_t + c_t, c_{t+1} = 0.5*c_t + 0.25 (exact dyadic): the
  reset -0.5*s_t = -0.25*sg_t - 0.25 splits into the PE matmul plus a
  constant folded into the per-step threshold. The PSUM bank is a pure
  matmul accumulation group and the DVE reads it as its single PSUM
  operand — every edge is a native RAW dependency (no foreign-data
  accumulation, which raced on cold runs in an earlier variant). PSUM
  holds exactly two chunks x two banks-in-flight (8 banks).

  DVE-only chunk (cols 2048:4096, carry neg_mem = -mem in SBUF):
      m_t       = stt(neg_mem, -0.5, x_t, mult, add)
      sg_t      = fp8(Sign(m_t - 1))            # ACT, output tile
      neg_mem_t = stt(m_t, 1.0, m_t, is_gt, subtract)   # (m>1) - m
  Its two stts are emitted ahead of the PE pair's in each step so the
  in-order DVE stream has work while ACT->PE produce the banks.

Each x half-tile has exactly one DMA writer (two DMAs into one tile
defeat the subtile dep tracker); loads alternate between the two HWDGE
queues (SP and ACT — the Pool queue is SWDGE and slower); spikes are
written into one wide [128, 4096] fp8 tile per step and leave as a
single coarse store per step on the Pool queue.

Sharding: batch dim B=32 (dim 1 after temporal expand) split across 8
cores, 4 per core -> per-core [T=8, 128 partitions, 4096 free] fp32.
"""

import numpy as np
import ml_dtypes

import concourse.bass as bass
import concourse.bacc as bacc
import concourse.tile as tile
from concourse import mybir
from concourse.bass_utils import run_bass_kernel_spmd

T = 8
B = 32
C = 128
H = 32
W = 32
NCORES = 8
BL = B // NCORES              # 4 batch elements per core
N = BL * C * H * W            # 524288 elements per timestep per core
P = 128                       # SBUF partitions
FREE = N // P                 # 4096 fp32 per partition per timestep
FCHUNK = 1024                 # PE-pair chunk width (2 PSUM banks fp32)
LCHUNK = 2048                 # load chunk / DVE-chunk width
MMF = 512                     # matmul moving free dim / PSUM bank width

_ALU = mybir.AluOpType

# Offset-membrane constants: c_0 = 0, c_{t+1} = 0.5*c_t + 0.25 (dyadic).
_CS = [0.0]
for _ in range(T - 1):
    _CS.append(0.5 * _CS[-1] + 0.25)
_THR = [-(1.0 + c) for c in _CS]   # ACT bias per step, PE-assisted pair


def build_bass(free: int = FREE):
    nc = bacc.Bacc("TRN2", target_bir_lowering=False, debug=False,
                   num_devices=NCORES)
    x_ap = nc.dram_tensor("x", [T, P, free], mybir.dt.float32,
                          kind="ExternalInput").ap()
    w_ap = nc.dram_tensor("w", [P, P], mybir.dt.float8e4,
                          kind="ExternalInput").ap()
    i_ap = nc.dram_tensor("i32", [P, P], mybir.dt.float32,
                          kind="ExternalInput").ap()
    o_ap = nc.dram_tensor("out", [T, P, free], mybir.dt.float8e4,
                          kind="ExternalOutput").ap()

    nhalf = free // LCHUNK
    pe_chunks = (0, 1)
    _F = mybir.ActivationFunctionType
    with tile.TileContext(nc) as tc:
        with (
            tc.tile_pool(name="xp", bufs=T) as xp,
            tc.tile_pool(name="sgp", bufs=6) as sgp,
            tc.tile_pool(name="mp", bufs=2) as mp,
            tc.tile_pool(name="pp", bufs=3) as pp,
            tc.psum_pool(name="qp", bufs=4) as qp,
            tc.tile_pool(name="cp", bufs=1) as cp,
        ):
            wt = cp.tile([P, P], mybir.dt.float8e4, tag="w")
            nc.sync.dma_start(wt[:], w_ap)
            it = cp.tile([P, P], mybir.dt.float32, tag="i32")
            nc.sync.dma_start(it[:], i_ap)
            biases = []
            for t in range(T):
                bt = cp.tile([P, 1], mybir.dt.float32, tag=f"b{t}")
                nc.gpsimd.memset(bt[:], _THR[t])
                biases.append(bt)

            # Preload x in t order, one tile per column-half (exactly one
            # DMA writer per tile): h0 tiles on the SP HWDGE ring, h1 on
            # the ACT HWDGE ring. The PE pipeline consumes x one step
            # ahead of the DVE chunk, so the SP ring carries one tile
            # less (x7h0 rides the ACT ring, before x7h1) and every
            # mid-stream h0 tile lands a slot earlier.
            xts = [[None] * nhalf for _ in range(T)]

            def load_half(t, li, eng):
                xt = xp.tile([P, LCHUNK], mybir.dt.float32, tag=f"x{li}")
                eng.dma_start(xt[:], x_ap[t, :, bass.ts(li, LCHUNK)])
                xts[t][li] = xt

            for t in range(T - 1):
                load_half(t, 0, nc.sync)
            for t in range(T - 1):
                load_half(t, 1, nc.scalar)
            load_half(T - 1, 0, nc.scalar)
            load_half(T - 1, 1, nc.scalar)

            def xsl(t, lo, width):
                half = lo // LCHUNK
                assert lo + width <= (half + 1) * LCHUNK
                return xts[t][half][:, bass.ds(lo % LCHUNK, width)]

            p = {c: xsl(0, c * FCHUNK, FCHUNK) for c in pe_chunks}
            bank = {}
            neg_mem = None
            for t in range(T):
                sgall = sgp.tile([P, free], mybir.dt.float8e4, tag="sg")
                # 1. PE prefetch: next-step banks start with I @ x_{t+1}
                if t < T - 1:
                    for c in pe_chunks:
                        qn = qp.tile([P, FCHUNK], mybir.dt.float32, tag="q")
                        for mi in range(FCHUNK // MMF):
                            nc.tensor.matmul(
                                qn[:, bass.ts(mi, MMF)], it[:],
                                xsl(t + 1, c * FCHUNK + mi * MMF, MMF),
                                start=True, stop=False)
                        bank[c] = qn
                # 2. DVE-only chunk first: keeps the in-order DVE stream
                #    busy while ACT/PE produce this step's banks
                if t == 0:
                    m_t = xsl(0, LCHUNK, LCHUNK)
                else:
                    mt = mp.tile([P, LCHUNK], mybir.dt.float32, tag="m")
                    nc.vector.scalar_tensor_tensor(
                        mt[:], neg_mem[:], -0.5, xsl(t, LCHUNK, LCHUNK),
                        _ALU.mult, _ALU.add)
                    m_t = mt[:]
                nc.scalar.activation(sgall[:, bass.ds(LCHUNK, LCHUNK)], m_t,
                                     _F.Sign, bias=biases[0][:])
                if t < T - 1:
                    nm = mp.tile([P, LCHUNK], mybir.dt.float32, tag="nm")
                    nc.vector.scalar_tensor_tensor(
                        nm[:], m_t, 1.0, m_t, _ALU.is_gt, _ALU.subtract)
                    neg_mem = nm
                # 3. PE pair: spike, correction matmul, membrane update
                for c in pe_chunks:
                    sg = sgall[:, bass.ts(c, FCHUNK)]
                    nc.scalar.activation(sg, p[c], _F.Sign,
                                         bias=biases[t][:])
                    if t < T - 1:
                        qn = bank[c]
                        for mi in range(FCHUNK // MMF):
                            nc.tensor.matmul(
                                qn[:, bass.ts(mi, MMF)], wt[:],
                                sgall[:, bass.ds(c * FCHUNK + mi * MMF, MMF)],
                                start=False, stop=True)
                        pn = pp.tile([P, FCHUNK], mybir.dt.float32, tag="p")
                        nc.vector.scalar_tensor_tensor(
                            pn[:], p[c], 0.5, qn[:], _ALU.mult, _ALU.add)
                        p[c] = pn[:]
                # 4. coarse store of the whole step's spikes on the HWDGE
                #    rings (they idle once loads drain; the Pool SWDGE ring
                #    added ~10us of post-compute store tail)
                eng = nc.sync if t % 2 == 0 else nc.scalar
                eng.dma_start(o_ap[t], sgall[:])
    nc.compile()
    return nc


_NC_CACHE: dict = {}


def _get_nc():
    if "nc" not in _NC_CACHE:
        _NC_CACHE["nc"] = build_bass()
    return _NC_CACHE["nc"]


def make_in_maps(x: np.ndarray):
    xs = x.reshape(T, B, C, H, W)
    w8 = (-0.25 * np.eye(P, dtype=np.float32)).astype(ml_dtypes.float8_e4m3)
    i32 = np.eye(P, dtype=np.float32)
    in_maps = []
    for i in range(NCORES):
        xi = np.ascontiguousarray(xs[:, i * BL:(i + 1) * BL])
        in_maps.append({"x": xi.reshape(T, P, FREE), "w": w8, "i32": i32})
    return in_maps


def kernel(x: np.ndarray) -> np.ndarray:
    x = np.asarray(x)
    assert x.shape == (T * B, C, H, W), x.shape
    in_dtype = x.dtype

    nc = _get_nc()
    res = run_bass_kernel_spmd(nc, make_in_maps(x), list(range(NCORES)))

    out = np.empty((T, B, C, H, W), dtype=np.float32)
    for i in range(NCORES):
        raw = np.asarray(res.results[i]["out"]).view(np.uint8)
        raw = raw.reshape(T, BL, C, H, W)
        # sg is {-1, 0, +1} in fp8e4m3; +1.0 encodes as byte 0x38
        out[:, i * BL:(i + 1) * BL] = (raw == 0x38)
    return out.reshape(T * B, C, H, W).astype(in_dtype, copy=False)


# revision 37
# speedup vs baseline: 1.0661x; 1.0661x over previous
"""LIF spiking-neuron kernel for Trainium2 (Bass/Tile), 8-core data-parallel.

Reference semantics (per element, scan over T=8):
    mem = mem * 0.5 + x_t
    s_t = (mem > 1.0) ? 1.0 : 0.0        # forward value of the spike
    mem = mem - s_t

Spikes leave the device as ONE byte per element: the ACT engine writes
sg_t = fp8e4(Sign(mem_t - thr)) in a single op ({-1,0,+1}; no Relu) and
the host decodes spikes as (byte == 0x38), the fp8 encoding of +1.0.
This cuts HBM store traffic 4x vs fp32 (16.78 MB in + 4.19 MB out per
core), so the Vector engine and the load stream set the pace.

The free axis (4096 fp32/partition) advances through the T=8 scan as
three column groups per step, split across two engine pipelines so the
Vector engine (the 2-tensor-op bottleneck at ~123 G elem/s) tracks the
DMA load rate:

  PE-assisted pair (cols 0:2048 as two 1024 chunks, membrane p in SBUF):
      PE:   bank_{t+1} = I @ x_{t+1}            # fp32 matmul, start=True
      ACT:  sg_t = fp8(Sign(p_t - (1+c_t)))     # also the output tile
      PE:   bank_{t+1} += (-0.25 I) @ sg_t      # fp8 matmul, stop=True
      DVE:  p_{t+1} = 0.5 * p_t + bank_{t+1}    # one stt (SBUF + PSUM in)
  where p_t = mem_t + c_t, c_{t+1} = 0.5*c_t + 0.25 (exact dyadic): the
  reset -0.5*s_t = -0.25*sg_t - 0.25 splits into the PE matmul plus a
  constant folded into the per-step threshold. The PSUM bank is a pure
  matmul accumulation group and the DVE reads it as its single PSUM
  operand — every edge is a native RAW dependency (no foreign-data
  accumulation, which raced on cold runs in an earlier variant). PSUM
  holds exactly two chunks x two banks-in-flight (8 banks).

  DVE-only chunk (cols 2048:4096, carry neg_mem = -mem in SBUF):
      m_t       = stt(neg_mem, -0.5, x_t, mult, add)
      sg_t      = fp8(Sign(m_t - 1))            # ACT, output tile
      neg_mem_t = stt(m_t, 1.0, m_t, is_gt, subtract)   # (m>1) - m
  Its two stts are emitted ahead of the PE pair's in each step so the
  in-order DVE stream has work while ACT->PE produce the banks.

Each x half-tile has exactly one DMA writer (two DMAs into one tile
defeat the subtile dep tracker); loads alternate between the two HWDGE
queues (SP and ACT — the Pool queue is SWDGE and slower); spikes are
written into one wide [128, 4096] fp8 tile per step and leave as a
single coarse store per step on the Pool queue.

Sharding: batch dim B=32 (dim 1 after temporal expand) split across 8
cores, 4 per core -> per-core [T=8, 128 partitions, 4096 free] fp32.
"""

import numpy as np
import ml_dtypes

import concourse.bass as bass
import concourse.bacc as bacc
import concourse.tile as tile
from concourse import mybir
from concourse.bass_utils import run_bass_kernel_spmd

T = 8
B = 32
C = 128
H = 32
W = 32
NCORES = 8
BL = B // NCORES              # 4 batch elements per core
N = BL * C * H * W            # 524288 elements per timestep per core
P = 128                       # SBUF partitions
FREE = N // P                 # 4096 fp32 per partition per timestep
FCHUNK = 1024                 # PE-pair chunk width (2 PSUM banks fp32)
LCHUNK = 2048                 # load chunk / DVE-chunk width
MMF = 512                     # matmul moving free dim / PSUM bank width

_ALU = mybir.AluOpType

# Offset-membrane constants: c_0 = 0, c_{t+1} = 0.5*c_t + 0.25 (dyadic).
_CS = [0.0]
for _ in range(T - 1):
    _CS.append(0.5 * _CS[-1] + 0.25)
_THR = [-(1.0 + c) for c in _CS]   # ACT bias per step, PE-assisted pair


def build_bass(free: int = FREE):
    nc = bacc.Bacc("TRN2", target_bir_lowering=False, debug=False,
                   num_devices=NCORES)
    x_ap = nc.dram_tensor("x", [T, P, free], mybir.dt.float32,
                          kind="ExternalInput").ap()
    w_ap = nc.dram_tensor("w", [P, P], mybir.dt.float8e4,
                          kind="ExternalInput").ap()
    i_ap = nc.dram_tensor("i32", [P, P], mybir.dt.float32,
                          kind="ExternalInput").ap()
    o_ap = nc.dram_tensor("out", [T, P, free], mybir.dt.float8e4,
                          kind="ExternalOutput").ap()

    nhalf = free // LCHUNK
    pe_chunks = (0, 1)
    _F = mybir.ActivationFunctionType
    with tile.TileContext(nc) as tc:
        with (
            tc.tile_pool(name="xp", bufs=T) as xp,
            tc.tile_pool(name="sgp", bufs=6) as sgp,
            tc.tile_pool(name="mp", bufs=2) as mp,
            tc.tile_pool(name="pp", bufs=3) as pp,
            tc.psum_pool(name="qp", bufs=4) as qp,
            tc.tile_pool(name="cp", bufs=1) as cp,
        ):
            wt = cp.tile([P, P], mybir.dt.float8e4, tag="w")
            nc.sync.dma_start(wt[:], w_ap)
            it = cp.tile([P, P], mybir.dt.float32, tag="i32")
            nc.sync.dma_start(it[:], i_ap)
            biases = []
            for t in range(T):
                bt = cp.tile([P, 1], mybir.dt.float32, tag=f"b{t}")
                nc.gpsimd.memset(bt[:], _THR[t])
                biases.append(bt)

            # Preload x in t order, one tile per column-half (exactly one
            # DMA writer per tile), halves alternating between the HWDGE
            # queues.
            xts = []
            for t in range(T):
                halves = []
                for li in range(nhalf):
                    xt = xp.tile([P, LCHUNK], mybir.dt.float32, tag=f"x{li}")
                    eng = nc.sync if (t * nhalf + li) % 2 == 0 else nc.scalar
                    eng.dma_start(xt[:], x_ap[t, :, bass.ts(li, LCHUNK)])
                    halves.append(xt)
                xts.append(halves)

            def xsl(t, lo, width):
                half = lo // LCHUNK
                assert lo + width <= (half + 1) * LCHUNK
                return xts[t][half][:, bass.ds(lo % LCHUNK, width)]

            p = {c: xsl(0, c * FCHUNK, FCHUNK) for c in pe_chunks}
            bank = {}
            neg_mem = None
            for t in range(T):
                sgall = sgp.tile([P, free], mybir.dt.float8e4, tag="sg")
                # 1. PE prefetch: next-step banks start with I @ x_{t+1}
                if t < T - 1:
                    for c in pe_chunks:
                        qn = qp.tile([P, FCHUNK], mybir.dt.float32, tag="q")
                        for mi in range(FCHUNK // MMF):
                            nc.tensor.matmul(
                                qn[:, bass.ts(mi, MMF)], it[:],
                                xsl(t + 1, c * FCHUNK + mi * MMF, MMF),
                                start=True, stop=False)
                        bank[c] = qn
                # 2. DVE-only chunk first: keeps the in-order DVE stream
                #    busy while ACT/PE produce this step's banks
                if t == 0:
                    m_t = xsl(0, LCHUNK, LCHUNK)
                else:
                    mt = mp.tile([P, LCHUNK], mybir.dt.float32, tag="m")
                    nc.vector.scalar_tensor_tensor(
                        mt[:], neg_mem[:], -0.5, xsl(t, LCHUNK, LCHUNK),
                        _ALU.mult, _ALU.add)
                    m_t = mt[:]
                nc.scalar.activation(sgall[:, bass.ds(LCHUNK, LCHUNK)], m_t,
                                     _F.Sign, bias=biases[0][:])
                if t < T - 1:
                    nm = mp.tile([P, LCHUNK], mybir.dt.float32, tag="nm")
                    nc.vector.scalar_tensor_tensor(
                        nm[:], m_t, 1.0, m_t, _ALU.is_gt, _ALU.subtract)
                    neg_mem = nm
                # 3. PE pair: spike, correction matmul, membrane update
                for c in pe_chunks:
                    sg = sgall[:, bass.ts(c, FCHUNK)]
                    nc.scalar.activation(sg, p[c], _F.Sign,
                                         bias=biases[t][:])
                    if t < T - 1:
                        qn = bank[c]
                        for mi in range(FCHUNK // MMF):
                            nc.tensor.matmul(
                                qn[:, bass.ts(mi, MMF)], wt[:],
                                sgall[:, bass.ds(c * FCHUNK + mi * MMF, MMF)],
                                start=False, stop=True)
                        pn = pp.tile([P, FCHUNK], mybir.dt.float32, tag="p")
                        nc.vector.scalar_tensor_tensor(
                            pn[:], p[c], 0.5, qn[:], _ALU.mult, _ALU.add)
                        p[c] = pn[:]
                # 4. coarse store of the whole step's spikes on the HWDGE
                #    rings (they idle once loads drain; the Pool SWDGE ring
                #    added ~10us of post-compute store tail)
                eng = nc.sync if t % 2 == 0 else nc.scalar
                eng.dma_start(o_ap[t], sgall[:])
    nc.compile()
    return nc


_NC_CACHE: dict = {}


def _get_nc():
    if "nc" not in _NC_CACHE:
        _NC_CACHE["nc"] = build_bass()
    return _NC_CACHE["nc"]


def make_in_maps(x: np.ndarray):
    xs = x.reshape(T, B, C, H, W)
    w8 = (-0.25 * np.eye(P, dtype=np.float32)).astype(ml_dtypes.float8_e4m3)
    i32 = np.eye(P, dtype=np.float32)
    in_maps = []
    for i in range(NCORES):
        xi = np.ascontiguousarray(xs[:, i * BL:(i + 1) * BL])
        in_maps.append({"x": xi.reshape(T, P, FREE), "w": w8, "i32": i32})
    return in_maps


def kernel(x: np.ndarray) -> np.ndarray:
    x = np.asarray(x)
    assert x.shape == (T * B, C, H, W), x.shape
    in_dtype = x.dtype

    nc = _get_nc()
    res = run_bass_kernel_spmd(nc, make_in_maps(x), list(range(NCORES)))

    out = np.empty((T, B, C, H, W), dtype=np.float32)
    for i in range(NCORES):
        raw = np.asarray(res.results[i]["out"]).view(np.uint8)
        raw = raw.reshape(T, BL, C, H, W)
        # sg is {-1, 0, +1} in fp8e4m3; +1.0 encodes as byte 0x38
        out[:, i * BL:(i + 1) * BL] = (raw == 0x38)
    return out.reshape(T * B, C, H, W).astype(in_dtype, copy=False)
